# revision 15
# baseline (speedup 1.0000x reference)
"""Trainium2 Bass kernel for nn_GATWithPool (dense 2-layer GAT + mean pool).

Sharding: destination-node dim (columns of the NxN maps) split across 8 cores,
256 columns each.  Each core streams its [12, 2048, 256] slab of attn_tensor
once (the memory-bound wall), builds agg/mask/edge-weights resident in SBUF,
and runs both GAT layers with the segment softmax folded into TensorE matmuls
(E[s,d] tiles as lhsT, [h | 1] as rhs -> numerator and denominator together).
Collectives: one AllGather (layer-1 output, transposed, + a_src2) and one
AllReduce (pooled graph features).
"""

import numpy as np
import concourse.bass as bass
import concourse.bacc as bacc
import concourse.mybir as mybir
import concourse.tile as tile
from concourse.bass_utils import run_bass_kernel_spmd

f32 = mybir.dt.float32
AF = mybir.ActivationFunctionType
ALU = mybir.AluOpType

N = 2048          # nodes
NCORE = 8
D = N // NCORE    # dst shard per core
P = 128           # partitions
T = N // P        # source tiles
KM = 12           # attention maps
H = 4             # layer-1 heads
C = 64            # per-head channels
HC = H * C        # 256
IN = 128          # input features
G = 8             # graphs
NOUT = 10
SLOPE = 0.2
PEN = 200.0       # additive mask penalty (exp(-~40) ~ 0)

# engine split for the 12-map agg accumulation (tuned from traces)
DVE_MAPS = list(range(0, 7))
POOL_MAPS = list(range(7, 12))


def build(n_cores=NCORE):
    nc = bacc.Bacc("TRN2", target_bir_lowering=False, debug=False,
                   num_devices=n_cores)

    attn_d = nc.declare_dram_parameter("attn", [KM, N, D], f32, isOutput=False)
    xt_d = nc.declare_dram_parameter("xt", [IN, N], f32, isOutput=False)
    xst_d = nc.declare_dram_parameter("xst", [IN, D], f32, isOutput=False)
    bcol_d = nc.declare_dram_parameter("bcol", [P, D], f32, isOutput=False)
    crow_d = nc.declare_dram_parameter("crow", [P, T], f32, isOutput=False)
    w1_d = nc.declare_dram_parameter("w1", [IN, HC], f32, isOutput=False)
    wasd1_d = nc.declare_dram_parameter("wasd1", [IN, 2 * H], f32, isOutput=False)
    w2_d = nc.declare_dram_parameter("w2", [2, P, C], f32, isOutput=False)
    wasd2_d = nc.declare_dram_parameter("wasd2", [2, P, 2], f32, isOutput=False)
    wlin_d = nc.declare_dram_parameter("wlin", [C, NOUT], f32, isOutput=False)
    bb1_d = nc.declare_dram_parameter("bb1", [P, HC], f32, isOutput=False)
    bb2_d = nc.declare_dram_parameter("bb2", [P, C], f32, isOutput=False)
    blin_d = nc.declare_dram_parameter("blin", [G, NOUT], f32, isOutput=False)
    inds_d = nc.declare_dram_parameter("inds", [2, P, G], f32, isOutput=False)
    ones_d = nc.declare_dram_parameter("ones128", [P, 1], f32, isOutput=False)
    wdb1_d = nc.declare_dram_parameter("wdb1", [IN, H * P], f32, isOutput=False)
    wdb2_d = nc.declare_dram_parameter("wdb2", [2, P, P], f32, isOutput=False)
    i128_d = nc.declare_dram_parameter("i128", [P, P], f32, isOutput=False)
    scal_d = nc.declare_dram_parameter("scal", [P, 32], f32, isOutput=False)

    ea_d = nc.declare_dram_parameter("edge_attr", [N, D], f32, isOutput=True)
    out_d = nc.declare_dram_parameter("out", [G, NOUT], f32, isOutput=True)

    with tile.TileContext(nc) as tc:
        from contextlib import ExitStack
        with ExitStack() as ctx:
            const = ctx.enter_context(tc.tile_pool(name="const", bufs=1))
            resid = ctx.enter_context(tc.tile_pool(name="resid", bufs=1))
            stage = ctx.enter_context(tc.tile_pool(name="stage", bufs=3))
            wpool = ctx.enter_context(tc.tile_pool(name="wpool", bufs=3))
            epool = ctx.enter_context(tc.tile_pool(name="epool", bufs=3))
            small = ctx.enter_context(tc.tile_pool(name="small", bufs=4))
            psP = ctx.enter_context(tc.tile_pool(name="psP", bufs=1, space="PSUM"))
            psT_ = ctx.enter_context(tc.tile_pool(name="psT", bufs=2, space="PSUM"))
            dram = ctx.enter_context(tc.tile_pool(name="dram", bufs=1, space="DRAM"))

            # ---- constants to SBUF ----
            xt = const.tile_from(xt_d[:])
            xst = const.tile_from(xst_d[:])
            bcol = const.tile_from(bcol_d[:])
            crow = const.tile_from(crow_d[:])
            w1 = const.tile_from(w1_d[:])
            wasd1 = const.tile_from(wasd1_d[:])
            wlin = const.tile_from(wlin_d[:])
            bb1 = const.tile_from(bb1_d[:])
            bb2 = const.tile_from(bb2_d[:])
            blin = const.tile_from(blin_d[:])
            ones128 = const.tile_from(ones_d[:])
            wdb1 = const.tile_from(wdb1_d[:])
            wdb2 = const.tile([P, 2 * P], f32)
            nc.sync.dma_start(wdb2[:].rearrange("p (k m) -> p k m", k=2),
                              wdb2_d[:].rearrange("k p m -> p k m"))
            i128 = const.tile_from(i128_d[:])
            scal = const.tile_from(scal_d[:])
            w2 = const.tile([P, 2 * C], f32)
            nc.sync.dma_start(w2[:].rearrange("p (k c) -> p k c", k=2),
                              w2_d[:].rearrange("k p c -> p k c"))
            wasd2 = const.tile([P, 4], f32)
            nc.sync.dma_start(wasd2[:].rearrange("p (k j) -> p k j", k=2),
                              wasd2_d[:].rearrange("k p j -> p k j"))
            inds = const.tile([P, 2 * G], f32)
            nc.sync.dma_start(inds[:].rearrange("p (b g) -> p b g", b=2),
                              inds_d[:].rearrange("b p g -> p b g"))

            # ---- resident tensors ----
            EV = resid.tile([P, T * D], f32)       # relu(agg), diag zeroed
            PE_ = resid.tile([P, T * D], f32)      # mask penalty {0,-1}
            HR = resid.tile([P, T * (H * 65)], f32)  # layer-1 rhs [h|1] per tile
            H2R = resid.tile([P, T * 65], f32)     # layer-2 rhs
            V2 = resid.tile([P, T * D], f32)       # layer-2 pre-lrelu logits
            h1T = resid.tile([P, 2 * N], f32)      # gathered h1 transposed
            h1s = resid.tile([P, 2 * HC], f32)     # local h1 shard [d, hc]
            h1Tl = resid.tile([P, 2 * D], f32)     # local h1 shard transposed
            Bad1 = resid.tile([P, H * D], f32)
            Bad2 = resid.tile([P, D], f32)
            asd1 = resid.tile([P, T * 8], f32)
            asd2c = resid.tile([P, T], f32)
            adT = resid.tile([8, D], f32)
            adC = resid.tile([P, 16], f32)
            ad2T = resid.tile([2, D], f32)
            ad2C = resid.tile([P, 4], f32)
            h_dsh = resid.tile([P, 2 * HC], f32)
            h2_dsh = resid.tile([P, 2 * C], f32)
            asd2l = resid.tile([P, 2], f32)
            mec = resid.tile([1, D], f32)
            meC = resid.tile([P, 2], f32)
            h2s = resid.tile([P, 2 * C], f32)
            gts = resid.tile([C, G], f32)
            gtf = resid.tile([C, G], f32)
            outf = resid.tile([G, NOUT], f32)

            # persistent PSUM accumulators
            psA = psP.tile([1, 2 * D], f32)            # colsums: [evsum | cnt]
            psB = [psP.tile([P, H * 65], f32, name=f"psB{i}") for i in range(2)]
            psL2 = psP.tile([P, 2 * 65], f32)          # L2 chains

            # pre-zero shared PSUM accumulators (chains use start=False so
            # interleaved accumulation groups in one bank don't zero each other)
            nc.vector.memset(psA[:], 0.0)
            nc.vector.memset(psB[0][:], 0.0)
            nc.vector.memset(psB[1][:], 0.0)
            nc.vector.memset(psL2[:], 0.0)

            # ones columns of the rhs tensors
            nc.gpsimd.memset(
                HR[:].rearrange("p (t h c) -> p t h c", t=T, h=H)[:, :, :, 64:65], 1.0)
            nc.gpsimd.memset(
                H2R[:].rearrange("p (t c) -> p t c", t=T)[:, :, 64:65], 1.0)

            # ---- phase 0: head projections / a_dst rows (from xsT) ----
            ps = psT_.tile([8, D], f32, tag="ps")
            nc.tensor.matmul(ps[:], wasd1[:], xst[:], start=True, stop=True)
            nc.scalar.copy(adT[:], ps[:])
            for blk in range(2):
                ps = psT_.tile([P, 8], f32, tag="ps")
                nc.tensor.matmul(ps[:], adT[:, blk * P:(blk + 1) * P],
                                 i128[0:8, 0:8], is_transpose=True,
                                 start=True, stop=True)
                nc.scalar.copy(adC[:, blk * 8:(blk + 1) * 8], ps[:])
            for h in range(H):
                ps = psT_.tile([P, D], f32, tag="ps")
                nc.tensor.matmul(ps[:], wdb1[:, h * P:(h + 1) * P], xst[:],
                                 start=True, stop=True)
                nc.scalar.copy(Bad1[:, h * D:(h + 1) * D], ps[:])
            for blk in range(2):
                ps = psT_.tile([P, HC], f32, tag="ps")
                nc.tensor.matmul(ps[:], xst[:, blk * P:(blk + 1) * P], w1[:],
                                 start=True, stop=True)
                nc.scalar.copy(h_dsh[:, blk * HC:(blk + 1) * HC], ps[:])

            # ---- phase 1: stream attn, build agg/EV/pen, layer-1 E + matmuls ----
            for t in range(T):
                sts = []
                for kc in range(3):
                    st = stage.tile([P, 4 * D], f32)
                    nc.sync.dma_start(
                        st[:].rearrange("p (k d) -> p k d", k=4),
                        attn_d[kc * 4:(kc + 1) * 4, t * P:(t + 1) * P, :]
                        .rearrange("k p d -> p k d"))
                    sts.append(st)

                def amap(k):
                    return sts[k // 4][:, (k % 4) * D:(k % 4 + 1) * D]

                aggA = wpool.tile([P, D], f32, tag="aggA")
                k0 = DVE_MAPS[0]
                nc.vector.tensor_scalar(aggA[:], amap(k0), scal[:, k0:k0 + 1],
                                        scal[:, 12:13], ALU.mult, ALU.add)
                for k in DVE_MAPS[1:]:
                    nc.vector.scalar_tensor_tensor(aggA[:], amap(k),
                                                   scal[:, k:k + 1], aggA[:],
                                                   ALU.mult, ALU.add)
                # Pool lacks STT/AP-scalar ops: ACT pre-scales, Pool adds.
                aggB = wpool.tile([P, D], f32, tag="aggB")
                k0 = POOL_MAPS[0]
                nc.scalar.activation(aggB[:], amap(k0), AF.Copy,
                                     scale=scal[:, k0:k0 + 1])
                for k in POOL_MAPS[1:]:
                    sc = wpool.tile([P, D], f32, tag="sc")
                    nc.scalar.activation(sc[:], amap(k), AF.Copy,
                                         scale=scal[:, k:k + 1])
                    nc.gpsimd.tensor_tensor(aggB[:], aggB[:], sc[:], ALU.add)
                agg = wpool.tile([P, D], f32, tag="agg")
                nc.vector.tensor_tensor(agg[:], aggA[:], aggB[:], ALU.add)

                ea = wpool.tile([P, D], f32, tag="ea")
                nc.scalar.activation(ea[:], agg[:], AF.Relu)
                nc.sync.dma_start(ea_d[t * P:(t + 1) * P, :], ea[:])

                ey = wpool.tile([P, D], f32, tag="ey")
                nc.vector.tensor_scalar(ey[:], bcol[:], crow[:, t:t + 1],
                                        None, ALU.is_equal)
                ne = wpool.tile([P, D], f32, tag="ne")
                nc.vector.tensor_scalar(ne[:], bcol[:], crow[:, t:t + 1],
                                        None, ALU.not_equal)

                EVt = EV[:, t * D:(t + 1) * D]
                PEt = PE_[:, t * D:(t + 1) * D]
                nc.vector.scalar_tensor_tensor(EVt, agg[:], 0.0, ne[:],
                                               ALU.max, ALU.mult)
                m01 = wpool.tile([P, D], f32, tag="m01")
                nc.vector.scalar_tensor_tensor(m01[:], agg[:], 0.0, ne[:],
                                               ALU.is_gt, ALU.mult)
                pn = wpool.tile([P, D], f32, tag="pn")
                nc.vector.scalar_tensor_tensor(pn[:], m01[:], 1.0, ey[:],
                                               ALU.subtract, ALU.add)
                nc.vector.tensor_scalar(PEt, pn[:], PEN, None, ALU.mult)

                nc.tensor.matmul(psA[0:1, 0:D], ones128[:], EVt,
                                 start=False, stop=(t == T - 1),
                                 skip_group_check=True)
                nc.tensor.matmul(psA[0:1, D:2 * D], ones128[:], m01[:],
                                 start=False, stop=(t == T - 1),
                                 skip_group_check=True)

                hps = psT_.tile([P, HC], f32, tag="ps")
                nc.tensor.matmul(hps[:], xt[:, t * P:(t + 1) * P], w1[:],
                                 start=True, stop=True)
                hr_t = HR[:, t * H * 65:(t + 1) * H * 65] \
                    .rearrange("p (h c) -> p h c", h=H)[:, :, 0:64]
                nc.scalar.copy(hr_t, hps[:].rearrange("p (h c) -> p h c", h=H))

                aps = psT_.tile([P, 8], f32, tag="ps")
                nc.tensor.matmul(aps[:], xt[:, t * P:(t + 1) * P], wasd1[:],
                                 start=True, stop=True)
                nc.scalar.copy(asd1[:, t * 8:(t + 1) * 8], aps[:])

                for h in range(H):
                    u = wpool.tile([P, D], f32, tag="u")
                    nc.vector.scalar_tensor_tensor(
                        u[:], EVt, scal[:, 13 + h:14 + h],
                        Bad1[:, h * D:(h + 1) * D], ALU.mult, ALU.add)
                    v = wpool.tile([P, D], f32, tag="v")
                    eng = nc.vector if h < 1 else nc.gpsimd
                    eng.tensor_tensor(v[:], u[:], PEt, ALU.add)
                    l = wpool.tile([P, D], f32, tag="l")
                    nc.scalar.activation(l[:], v[:], AF.Prelu,
                                         bias=asd1[:, t * 8 + h:t * 8 + h + 1],
                                         alpha=SLOPE)
                    e = epool.tile([P, D], f32, tag="e")
                    nc.scalar.activation(e[:], l[:], AF.Exp)
                    for blk in range(2):
                        nc.tensor.matmul(
                            psB[blk][:, h * 65:(h + 1) * 65],
                            e[:, blk * P:(blk + 1) * P],
                            HR[:, t * H * 65 + h * 65:t * H * 65 + (h + 1) * 65],
                            start=False, stop=(t == T - 1),
                            skip_group_check=True)

            # ---- phase 2: mean_e, L1 diag corrections, h1, transpose, AG ----
            c1 = small.tile([1, D], f32, tag="c1")
            nc.vector.tensor_scalar(c1[:], psA[0:1, D:2 * D], 1.0, None, ALU.max)
            rci = small.tile([1, D], f32, tag="rci")
            nc.vector.reciprocal(rci[:], c1[:])
            nc.vector.tensor_tensor(mec[:], psA[0:1, 0:D], rci[:], ALU.mult)
            for blk in range(2):
                ps = psT_.tile([P, 1], f32, tag="ps")
                nc.tensor.matmul(ps[:], mec[0:1, blk * P:(blk + 1) * P],
                                 i128[0:1, 0:1], is_transpose=True,
                                 start=True, stop=True)
                nc.scalar.copy(meC[:, blk:blk + 1], ps[:])

            for h in range(H):
                for blk in range(2):
                    asum = small.tile([P, 1], f32, tag="asum")
                    nc.vector.tensor_tensor(
                        asum[:], adC[:, blk * 8 + h:blk * 8 + h + 1],
                        adC[:, blk * 8 + 4 + h:blk * 8 + 5 + h], ALU.add)
                    vt = small.tile([P, 1], f32, tag="vt")
                    nc.vector.scalar_tensor_tensor(
                        vt[:], meC[:, blk:blk + 1], scal[:, 13 + h:14 + h],
                        asum[:], ALU.mult, ALU.add)
                    et = small.tile([P, 1], f32, tag="et")
                    nc.scalar.activation(et[:], vt[:], AF.Prelu, alpha=SLOPE)
                    nc.scalar.activation(et[:], et[:], AF.Exp)
                    ew = small.tile([P, 1], f32, tag="ew")
                    nc.scalar.activation(ew[:], asum[:], AF.Prelu, alpha=SLOPE)
                    nc.scalar.activation(ew[:], ew[:], AF.Exp)
                    dE = small.tile([P, 1], f32, tag="dE")
                    nc.vector.tensor_tensor(dE[:], et[:], ew[:], ALU.subtract)
                    pb = psB[blk]
                    nc.vector.scalar_tensor_tensor(
                        pb[:, h * 65:h * 65 + 64],
                        h_dsh[:, blk * HC + h * C:blk * HC + (h + 1) * C],
                        dE[:], pb[:, h * 65:h * 65 + 64], ALU.mult, ALU.add)
                    nc.vector.tensor_tensor(
                        pb[:, h * 65 + 64:h * 65 + 65],
                        pb[:, h * 65 + 64:h * 65 + 65], dE[:], ALU.add)

            # normalize + bias + ELU -> h1 shard [d, hc]
            for blk in range(2):
                pb = psB[blk]
                for h in range(H):
                    rc = small.tile([P, 1], f32, tag="rc")
                    nc.vector.reciprocal(rc[:], pb[:, h * 65 + 64:h * 65 + 65])
                    nc.vector.tensor_scalar(
                        h1s[:, blk * HC + h * C:blk * HC + (h + 1) * C],
                        pb[:, h * 65:h * 65 + 64], rc[:], None, ALU.mult)
                hb = h1s[:, blk * HC:(blk + 1) * HC]
                nc.vector.tensor_tensor(hb, hb, bb1[:], ALU.add)
                mn = wpool.tile([P, HC], f32, tag="mn")
                nc.vector.tensor_scalar(mn[:], hb, 0.0, None, ALU.min)
                nc.scalar.activation(mn[:], mn[:], AF.Exp)
                nc.vector.tensor_scalar(mn[:], mn[:], 1.0, None, ALU.subtract)
                rl = wpool.tile([P, HC], f32, tag="rl")
                nc.scalar.activation(rl[:], hb, AF.Relu)
                nc.vector.tensor_tensor(hb, rl[:], mn[:], ALU.add)

            for blk in range(2):
                for kb in range(2):
                    ps = psT_.tile([P, P], f32, tag="ps")
                    nc.tensor.matmul(ps[:],
                                     h1s[:, blk * HC + kb * P:blk * HC + (kb + 1) * P],
                                     i128[:], is_transpose=True,
                                     start=True, stop=True)
                    nc.scalar.copy(h1Tl[:, kb * D + blk * P:kb * D + (blk + 1) * P],
                                   ps[:])

            # a_src2 for own rows; a_dst2 row; h2 pre-acts for own rows
            for blk in range(2):
                ps = psT_.tile([P, 2], f32, tag="ps")
                for kt in range(2):
                    nc.tensor.matmul(ps[:],
                                     h1Tl[:, kt * D + blk * P:kt * D + (blk + 1) * P],
                                     wasd2[:, kt * 2:(kt + 1) * 2],
                                     start=(kt == 0), stop=(kt == 1))
                nc.scalar.copy(asd2l[:, blk:blk + 1], ps[:, 0:1])
                ps2 = psT_.tile([P, C], f32, tag="ps")
                for kt in range(2):
                    nc.tensor.matmul(ps2[:],
                                     h1Tl[:, kt * D + blk * P:kt * D + (blk + 1) * P],
                                     w2[:, kt * C:(kt + 1) * C],
                                     start=(kt == 0), stop=(kt == 1))
                nc.scalar.copy(h2_dsh[:, blk * C:(blk + 1) * C], ps2[:])

            ps = psT_.tile([2, D], f32, tag="ps")
            for kt in range(2):
                nc.tensor.matmul(ps[:], wasd2[:, kt * 2:(kt + 1) * 2],
                                 h1Tl[:, kt * D:(kt + 1) * D],
                                 start=(kt == 0), stop=(kt == 1))
            nc.scalar.copy(ad2T[:], ps[:])
            ps = psT_.tile([P, D], f32, tag="ps")
            for kt in range(2):
                nc.tensor.matmul(ps[:], wdb2[:, kt * P:(kt + 1) * P],
                                 h1Tl[:, kt * D:(kt + 1) * D],
                                 start=(kt == 0), stop=(kt == 1))
            nc.scalar.copy(Bad2[:], ps[:])
            for blk in range(2):
                ps = psT_.tile([P, 2], f32, tag="ps")
                nc.tensor.matmul(ps[:], ad2T[:, blk * P:(blk + 1) * P],
                                 i128[0:2, 0:2], is_transpose=True,
                                 start=True, stop=True)
                nc.scalar.copy(ad2C[:, blk * 2:(blk + 1) * 2], ps[:])

            # AllGather payload: rows (kt,p) -> h1T block; col 256 -> a_src2
            agin = dram.tile([2 * P, D + 1], f32)
            agout = dram.tile([NCORE * 2 * P, D + 1], f32)
            for kt in range(2):
                nc.sync.dma_start(agin[kt * P:(kt + 1) * P, 0:D],
                                  h1Tl[:, kt * D:(kt + 1) * D])
            for blk in range(2):
                nc.sync.dma_start(agin[blk * P:(blk + 1) * P, D:D + 1],
                                  asd2l[:, blk:blk + 1])
            nc.gpsimd.collective_compute(
                "AllGather", ALU.bypass,
                replica_groups=[list(range(NCORE))],
                ins=[agin.opt()], outs=[agout.opt()])

            # ---- phase 3 (overlaps AG): layer-2 logits from EV/pen ----
            for t in range(T):
                u2 = wpool.tile([P, D], f32, tag="u2")
                nc.vector.scalar_tensor_tensor(u2[:], EV[:, t * D:(t + 1) * D],
                                               scal[:, 17:18], Bad2[:],
                                               ALU.mult, ALU.add)
                nc.gpsimd.tensor_tensor(V2[:, t * D:(t + 1) * D], u2[:],
                                        PE_[:, t * D:(t + 1) * D], ALU.add)

            # ---- phase 4: post-AG layer-2 ----
            for kt in range(2):
                nc.sync.dma_start(
                    h1T[:, kt * N:(kt + 1) * N].rearrange("p (c j) -> p c j", c=NCORE),
                    agout[:, 0:D].rearrange("(c k p) j -> k p c j", c=NCORE, k=2)[kt])
            nc.sync.dma_start(
                asd2c[:],
                agout[:, D:D + 1].rearrange("(t p) o -> p (t o)", p=P))

            for t in range(T):
                ps = psT_.tile([P, C], f32, tag="ps")
                for kt in range(2):
                    nc.tensor.matmul(ps[:],
                                     h1T[:, kt * N + t * P:kt * N + (t + 1) * P],
                                     w2[:, kt * C:(kt + 1) * C],
                                     start=(kt == 0), stop=(kt == 1))
                nc.scalar.copy(H2R[:, t * 65:t * 65 + 64], ps[:])

                l2 = wpool.tile([P, D], f32, tag="l2")
                nc.scalar.activation(l2[:], V2[:, t * D:(t + 1) * D], AF.Prelu,
                                     bias=asd2c[:, t:t + 1], alpha=SLOPE)
                e2 = epool.tile([P, D], f32, tag="e2")
                nc.scalar.activation(e2[:], l2[:], AF.Exp)
                for blk in range(2):
                    nc.tensor.matmul(psL2[:, blk * 65:(blk + 1) * 65],
                                     e2[:, blk * P:(blk + 1) * P],
                                     H2R[:, t * 65:(t + 1) * 65],
                                     start=False, stop=(t == T - 1),
                                     skip_group_check=True)

            # ---- phase 5: L2 diag correction, h2, pool, AllReduce, linear ----
            for blk in range(2):
                asum = small.tile([P, 1], f32, tag="asum2")
                nc.vector.tensor_tensor(asum[:], ad2C[:, blk * 2:blk * 2 + 1],
                                        ad2C[:, blk * 2 + 1:blk * 2 + 2], ALU.add)
                vt = small.tile([P, 1], f32, tag="vt2")
                nc.vector.scalar_tensor_tensor(vt[:], meC[:, blk:blk + 1],
                                               scal[:, 17:18], asum[:],
                                               ALU.mult, ALU.add)
                et = small.tile([P, 1], f32, tag="et2")
                nc.scalar.activation(et[:], vt[:], AF.Prelu, alpha=SLOPE)
                nc.scalar.activation(et[:], et[:], AF.Exp)
                ew = small.tile([P, 1], f32, tag="ew2")
                nc.scalar.activation(ew[:], asum[:], AF.Prelu, alpha=SLOPE)
                nc.scalar.activation(ew[:], ew[:], AF.Exp)
                dE = small.tile([P, 1], f32, tag="dE2")
                nc.vector.tensor_tensor(dE[:], et[:], ew[:], ALU.subtract)
                nc.vector.scalar_tensor_tensor(
                    psL2[:, blk * 65:blk * 65 + 64],
                    h2_dsh[:, blk * C:(blk + 1) * C], dE[:],
                    psL2[:, blk * 65:blk * 65 + 64], ALU.mult, ALU.add)
                nc.vector.tensor_tensor(psL2[:, blk * 65 + 64:blk * 65 + 65],
                                        psL2[:, blk * 65 + 64:blk * 65 + 65],
                                        dE[:], ALU.add)

                rc = small.tile([P, 1], f32, tag="rc2")
                nc.vector.reciprocal(rc[:], psL2[:, blk * 65 + 64:blk * 65 + 65])
                hb = h2s[:, blk * C:(blk + 1) * C]
                nc.vector.tensor_scalar(hb, psL2[:, blk * 65:blk * 65 + 64],
                                        rc[:], None, ALU.mult)
                nc.vector.tensor_tensor(hb, hb, bb2[:], ALU.add)
                mn = wpool.tile([P, C], f32, tag="mn2")
                nc.vector.tensor_scalar(mn[:], hb, 0.0, None, ALU.min)
                nc.scalar.activation(mn[:], mn[:], AF.Exp)
                nc.vector.tensor_scalar(mn[:], mn[:], 1.0, None, ALU.subtract)
                rl = wpool.tile([P, C], f32, tag="rl2")
                nc.scalar.activation(rl[:], hb, AF.Relu)
                nc.vector.tensor_tensor(hb, rl[:], mn[:], ALU.add)

            psGT = psT_.tile([C, G], f32, tag="ps")
            for blk in range(2):
                nc.tensor.matmul(psGT[:], h2s[:, blk * C:(blk + 1) * C],
                                 inds[:, blk * G:(blk + 1) * G],
                                 start=(blk == 0), stop=(blk == 1))
            nc.scalar.copy(gts[:], psGT[:])
            arin = dram.tile([C, G], f32)
            arout = dram.tile([C, G], f32)
            nc.sync.dma_start(arin[:], gts[:])
            nc.gpsimd.collective_compute(
                "AllReduce", ALU.add,
                replica_groups=[list(range(NCORE))],
                ins=[arin.opt()], outs=[arout.opt()])
            nc.sync.dma_start(gtf[:], arout[:])
            psO = psT_.tile([G, NOUT], f32, tag="ps")
            nc.tensor.matmul(psO[:], gtf[:], wlin[:], start=True, stop=True)
            nc.vector.tensor_tensor(outf[:], psO[:], blin[:], ALU.add)
            nc.sync.dma_start(out_d[:], outf[:])

    nc.compile()
    return nc


_NC_CACHE = {}


def _get_nc():
    if "nc" not in _NC_CACHE:
        _NC_CACHE["nc"] = build()
    return _NC_CACHE["nc"]


def make_in_maps(x, attn_tensor, batch_idx, w_agg, b_agg, W1, att_src1,
                 att_dst1, We1, att_e1, bias1, W2, att_src2, att_dst2, We2,
                 att_e2, bias2, W_lin, b_lin):
    x = np.asarray(x, np.float32)
    attn = np.asarray(attn_tensor, np.float32)
    bidx = np.asarray(batch_idx).astype(np.int64)
    W1 = np.asarray(W1, np.float32)
    W2 = np.asarray(W2, np.float32)

    wasd1 = np.empty((IN, 2 * H), np.float32)
    for h in range(H):
        wasd1[:, h] = W1[:, h * C:(h + 1) * C] @ np.asarray(att_src1, np.float32)[h]
        wasd1[:, 4 + h] = W1[:, h * C:(h + 1) * C] @ np.asarray(att_dst1, np.float32)[h]
    we1 = (np.asarray(We1, np.float32).reshape(H, C)
           * np.asarray(att_e1, np.float32)).sum(-1)
    we2 = float((np.asarray(We2, np.float32).reshape(1, C)
                 * np.asarray(att_e2, np.float32)).sum())
    wasd2 = np.stack([W2 @ np.asarray(att_src2, np.float32)[0],
                      W2 @ np.asarray(att_dst2, np.float32)[0]], axis=1)

    scal = np.zeros((P, 32), np.float32)
    scal[:, 0:KM] = np.asarray(w_agg, np.float32)[None, :]
    scal[:, 12] = np.float32(b_agg)
    scal[:, 13:13 + H] = we1[None, :]
    scal[:, 17] = we2

    cnt = np.bincount(bidx, minlength=G).astype(np.float32)
    cntc = np.maximum(cnt, 1.0)
    onehot = (bidx[:, None] == np.arange(G)[None, :]).astype(np.float32) / cntc[None, :]

    crow = (np.arange(T)[None, :] * P + np.arange(P)[:, None]).astype(np.float32)

    common = {
        "xt": np.ascontiguousarray(x.T),
        "crow": crow,
        "w1": W1,
        "wasd1": wasd1,
        "w2": np.ascontiguousarray(W2.reshape(2, P, C)),
        "wasd2": np.ascontiguousarray(wasd2.reshape(2, P, 2)),
        "wlin": np.asarray(W_lin, np.float32),
        "bb1": np.tile(np.asarray(bias1, np.float32)[None, :], (P, 1)),
        "bb2": np.tile(np.asarray(bias2, np.float32)[None, :], (P, 1)),
        "blin": np.tile(np.asarray(b_lin, np.float32)[None, :], (G, 1)),
        "ones128": np.ones((P, 1), np.float32),
        "wdb1": np.repeat(wasd1[:, 4:8], P, axis=1).reshape(IN, H, P).reshape(IN, H * P),
        "wdb2": np.ascontiguousarray(
            np.tile(wasd2.reshape(2, P, 2)[:, :, 1:2], (1, 1, P))),
        "i128": np.eye(P, dtype=np.float32),
        "scal": scal,
    }
    in_maps = []
    for c in range(NCORE):
        m = dict(common)
        m["attn"] = np.ascontiguousarray(attn[:, :, c * D:(c + 1) * D])
        m["xst"] = np.ascontiguousarray(x[c * D:(c + 1) * D, :].T)
        m["bcol"] = np.tile(np.arange(c * D, (c + 1) * D, dtype=np.float32),
                            (P, 1))
        m["inds"] = np.ascontiguousarray(
            onehot[c * D:(c + 1) * D].reshape(2, P, G))
        in_maps.append(m)
    return in_maps


def run(in_maps, **kwargs):
    nc = _get_nc()
    return run_bass_kernel_spmd(nc, in_maps, list(range(NCORE)), **kwargs)


def kernel(**inputs):
    in_maps = make_in_maps(**inputs)
    res = run(in_maps)
    edge_attr = np.concatenate(
        [res.results[c]["edge_attr"] for c in range(NCORE)], axis=1)
    out = res.results[0]["out"]
    return out.astype(np.float32), edge_attr.astype(np.float32)
